# revision 1
# baseline (speedup 1.0000x reference)
"""BitConv2dInfer on 8 Trainium2 NeuronCores.

Reference computation (per full input):
    x = clip(x, -1, 1)                       # x [32, 256, 56, 56] f32
    y = conv2d(x, w_q, pad=1)                # w_q [256, 256, 3, 3] ternary
    y = y * s + bias                         # per-out-channel affine
Sharding: data-parallel over batch — each of the 8 cores gets 4 images and
the full (tiny) weights; outputs concatenate over batch with no comms.

Device kernel (per core, per image):
  - DMA x[n] in as 2 CIN tiles of [128, 56, 56] f32
  - clamp to [-1,1] + cast to bf16 into a zero-bordered [128, 58, 58] pad tile
  - conv as 18 accumulated PE matmuls per (cout_tile, 8-row chunk):
      psum[co*128+m, oh, ow] += sum_k w[k, co*128+m] * xpad[k, oh+kh, ow+kw]
    lhsT = w slice [128 cin, 128 cout], rhs = shifted pad-tile window [128, 8, 56]
  - scalar-engine activation evacuates PSUM with per-partition scale+bias
  - DMA [128, 56, 56] f32 result tiles back out

Weights are host-side transposed to lhsT layout [128 cin, (tap, ci, cout)] and
cast to bf16 (exact for ternary values).
"""

import sys

sys.path.insert(0, "/opt/trn_rl_repo")

import ml_dtypes
import numpy as np

import concourse.bass as bass  # noqa: F401  (registers engines)
import concourse.mybir as mybir
import concourse.tile as tile
from concourse import bacc
from concourse.bass_utils import run_bass_kernel_spmd

N, CIN, COUT, H, W = 32, 256, 256, 56, 56
NCORES = 8
NB = N // NCORES          # images per core
HP, WP = H + 2, W + 2     # padded spatial
RG = 8                    # output rows per PSUM chunk (8*56=448 <= 512 f32/bank)
NCH = H // RG             # chunks per image
NCI = CIN // 128          # cin tiles
NCO = COUT // 128         # cout tiles
NTAP = 9

_compiled = {}


def _build():
    nc = bacc.Bacc("TRN2", target_bir_lowering=False, debug=False)
    f32, bf16 = mybir.dt.float32, mybir.dt.bfloat16
    x_d = nc.dram_tensor("x", [NB, CIN, H, W], f32, kind="ExternalInput").ap()
    w_d = nc.dram_tensor("w", [128, NTAP * NCI * COUT], bf16, kind="ExternalInput").ap()
    s_d = nc.dram_tensor("s", [128, NCO], f32, kind="ExternalInput").ap()
    b_d = nc.dram_tensor("b", [128, NCO], f32, kind="ExternalInput").ap()
    o_d = nc.dram_tensor("out", [NB, COUT, H, W], f32, kind="ExternalOutput").ap()

    with tile.TileContext(nc) as tc:
        with (
            tc.tile_pool(name="const", bufs=1) as cpool,
            tc.tile_pool(name="xs", bufs=3) as xspool,
            tc.tile_pool(name="xpad", bufs=2) as xppool,
            tc.tile_pool(name="osb", bufs=3) as opool,
            tc.tile_pool(name="ps", bufs=4, space="PSUM") as pspool,
        ):
            w_sb = cpool.tile([128, NTAP * NCI * COUT], bf16, tag="w")
            s_sb = cpool.tile([128, NCO], f32, tag="s")
            b_sb = cpool.tile([128, NCO], f32, tag="b")
            nc.sync.dma_start(out=w_sb[:], in_=w_d)
            nc.sync.dma_start(out=s_sb[:], in_=s_d)
            nc.sync.dma_start(out=b_sb[:], in_=b_d)

            for n in range(NB):
                xp = []
                for ci in range(NCI):
                    xs = xspool.tile([128, H, W], f32, tag="xs")
                    nc.sync.dma_start(out=xs[:], in_=x_d[n, ci * 128:(ci + 1) * 128])
                    xpad = xppool.tile([128, HP, WP], bf16, tag=f"xpad{ci}")
                    nc.vector.memset(xpad[:, 0:1, :], 0.0)
                    nc.vector.memset(xpad[:, HP - 1:HP, :], 0.0)
                    nc.vector.memset(xpad[:, 1:HP - 1, 0:1], 0.0)
                    nc.vector.memset(xpad[:, 1:HP - 1, WP - 1:WP], 0.0)
                    nc.vector.tensor_scalar(
                        xpad[:, 1:H + 1, 1:W + 1], xs[:],
                        -1.0, 1.0, mybir.AluOpType.max, mybir.AluOpType.min,
                    )
                    xp.append(xpad)
                for co in range(NCO):
                    osb = opool.tile([128, H, W], f32, tag="osb")
                    for c in range(NCH):
                        ps = pspool.tile([128, RG, W], f32, tag="ps")
                        for t in range(NTAP):
                            kh, kw = divmod(t, 3)
                            for ci in range(NCI):
                                wcol = (t * NCI + ci) * COUT + co * 128
                                nc.tensor.matmul(
                                    out=ps[:],
                                    lhsT=w_sb[:, wcol:wcol + 128],
                                    rhs=xp[ci][:, c * RG + kh:c * RG + kh + RG, kw:kw + W],
                                    start=(t == 0 and ci == 0),
                                    stop=(t == NTAP - 1 and ci == NCI - 1),
                                )
                        nc.scalar.activation(
                            out=osb[:, c * RG:(c + 1) * RG, :], in_=ps[:],
                            func=mybir.ActivationFunctionType.Identity,
                            bias=b_sb[:, co:co + 1], scale=s_sb[:, co:co + 1],
                        )
                    nc.sync.dma_start(out=o_d[n, co * 128:(co + 1) * 128], in_=osb[:])

    nc.compile()
    return nc


def _prep_weights(w_q, s, bias):
    # lhsT layout: [cin_k (128 partitions), tap, ci, cout] so that
    # lhsT[k, t, ci, j] = w_q[j, ci*128 + k, kh, kw]
    w_t = (
        w_q.astype(np.float32)
        .transpose(2, 3, 1, 0)                 # [kh, kw, CIN, COUT]
        .reshape(NTAP, NCI, 128, COUT)         # [tap, ci, k, cout]
        .transpose(2, 0, 1, 3)                 # [k, tap, ci, cout]
        .reshape(128, NTAP * NCI * COUT)
        .astype(ml_dtypes.bfloat16)
    )
    s_t = np.ascontiguousarray(s.reshape(NCO, 128).T.astype(np.float32))
    b_t = np.ascontiguousarray(bias.reshape(NCO, 128).T.astype(np.float32))
    return w_t, s_t, b_t


def kernel(x, w_q, s, bias):
    if "nc" not in _compiled:
        _compiled["nc"] = _build()
    nc = _compiled["nc"]

    w_t, s_t, b_t = _prep_weights(w_q, s, bias)
    x = np.ascontiguousarray(x, dtype=np.float32)
    core_ids = list(range(NCORES))
    in_maps = [
        {"x": x[i * NB:(i + 1) * NB], "w": w_t, "s": s_t, "b": b_t}
        for i in core_ids
    ]
    res = run_bass_kernel_spmd(nc, in_maps, core_ids)
    return np.concatenate([res.results[i]["out"] for i in core_ids], axis=0)


# revision 2
# speedup vs baseline: 1.0330x; 1.0330x over previous
"""BitConv2dInfer on 8 Trainium2 NeuronCores.

Reference computation (per full input):
    x = clip(x, -1, 1)                       # x [32, 256, 56, 56] f32
    y = conv2d(x, w_q, pad=1)                # w_q [256, 256, 3, 3] ternary
    y = y * s + bias                         # per-out-channel affine
Sharding: data-parallel over batch — each of the 8 cores gets 4 images and
the full (tiny) weights; outputs concatenate over batch with no comms.

Device kernel (per core, per image):
  - DMA x[n] in as 2 CIN tiles of [128, 56, 56] f32 (row-chunked for the
    first image so the PE can start before the full image lands)
  - clamp to [-1,1] + cast to bf16 into a zero-bordered [128, 58, 58] pad tile
  - conv as 18 accumulated PE matmuls per (cout_tile, 8-row chunk):
      psum[co*128+m, oh, ow] += sum_k w[k, co*128+m] * xpad[k, oh+kh, ow+kw]
    lhsT = w slice [128 cin, 128 cout], rhs = shifted pad-tile window [128, 8, 56]
  - scalar-engine activation evacuates PSUM with per-partition scale+bias
  - DMA [128, 56, 56] f32 result tiles back out (split in two so the tail
    drains early)

Weights are host-side transposed to lhsT layout [128 cin, co, (tap, ci), cout]
and cast to bf16 (exact for ternary values).
"""

import sys

sys.path.insert(0, "/opt/trn_rl_repo")

import ml_dtypes
import numpy as np

import concourse.bass as bass  # noqa: F401  (registers engines)
import concourse.mybir as mybir
import concourse.tile as tile
from concourse import bacc
from concourse.bass_utils import run_bass_kernel_spmd

N, CIN, COUT, H, W = 32, 256, 256, 56, 56
NCORES = 8
NB = N // NCORES          # images per core
HP, WP = H + 2, W + 2     # padded spatial
RG = 8                    # output rows per PSUM chunk (8*56=448 <= 512 f32/bank)
NCH = H // RG             # chunks per image
NCI = CIN // 128          # cin tiles
NCO = COUT // 128         # cout tiles
NTAP = 9

_compiled = {}


def _build():
    nc = bacc.Bacc("TRN2", target_bir_lowering=False, debug=False)
    f32, bf16 = mybir.dt.float32, mybir.dt.bfloat16
    x_d = nc.dram_tensor("x", [NB, CIN, H, W], f32, kind="ExternalInput").ap()
    w_d = nc.dram_tensor(
        "w", [128, NCO, NTAP * NCI, 128], bf16, kind="ExternalInput"
    ).ap()
    s_d = nc.dram_tensor("s", [128, NCO], f32, kind="ExternalInput").ap()
    b_d = nc.dram_tensor("b", [128, NCO], f32, kind="ExternalInput").ap()
    o_d = nc.dram_tensor("out", [NB, COUT, H, W], f32, kind="ExternalOutput").ap()

    clamp = dict(op0=mybir.AluOpType.max, op1=mybir.AluOpType.min)

    with tile.TileContext(nc) as tc:
        with (
            tc.tile_pool(name="const", bufs=1) as cpool,
            tc.tile_pool(name="xs", bufs=3) as xspool,
            tc.tile_pool(name="xsc", bufs=4) as xscpool,
            tc.tile_pool(name="xpad", bufs=2) as xppool,
            tc.tile_pool(name="osb", bufs=3) as opool,
            tc.tile_pool(name="ps", bufs=4, space="PSUM") as pspool,
        ):
            w_sb = cpool.tile([128, NCO, NTAP * NCI, 128], bf16, tag="w")
            s_sb = cpool.tile([128, NCO], f32, tag="s")
            b_sb = cpool.tile([128, NCO], f32, tag="b")
            # co=0 weights land first: the first matmul groups need only them.
            nc.sync.dma_start(out=w_sb[:, 0], in_=w_d[:, 0])
            nc.sync.dma_start(out=w_sb[:, 1], in_=w_d[:, 1])
            nc.sync.dma_start(out=s_sb[:], in_=s_d)
            nc.sync.dma_start(out=b_sb[:], in_=b_d)

            for n in range(NB):
                xp = []
                for ci in range(NCI):
                    xpad = xppool.tile([128, HP, WP], bf16, tag=f"xpad{ci}")
                    nc.vector.memset(xpad[:, 0:1, :], 0.0)
                    nc.vector.memset(xpad[:, HP - 1:HP, :], 0.0)
                    nc.vector.memset(xpad[:, 1:HP - 1, 0:1], 0.0)
                    nc.vector.memset(xpad[:, 1:HP - 1, WP - 1:WP], 0.0)
                    src = x_d[n, ci * 128:(ci + 1) * 128]
                    if n == 0:
                        # Row-chunked load+clamp so group c (which reads padded
                        # rows [8c, 8c+10)) only waits for chunks c-1..c+1.
                        for c in range(NCH):
                            xs = xscpool.tile([128, RG, W], f32, tag=f"xsc{ci}")
                            nc.sync.dma_start(
                                out=xs[:], in_=src[:, c * RG:(c + 1) * RG]
                            )
                            nc.vector.tensor_scalar(
                                xpad[:, c * RG + 1:(c + 1) * RG + 1, 1:W + 1],
                                xs[:], -1.0, 1.0, **clamp,
                            )
                    else:
                        xs = xspool.tile([128, H, W], f32, tag="xs")
                        nc.sync.dma_start(out=xs[:], in_=src)
                        nc.vector.tensor_scalar(
                            xpad[:, 1:H + 1, 1:W + 1], xs[:], -1.0, 1.0, **clamp
                        )
                    xp.append(xpad)
                for co in range(NCO):
                    osb = opool.tile([128, H, W], f32, tag="osb")
                    for c in range(NCH):
                        ps = pspool.tile([128, RG, W], f32, tag="ps")
                        for t in range(NTAP):
                            kh, kw = divmod(t, 3)
                            for ci in range(NCI):
                                nc.tensor.matmul(
                                    out=ps[:],
                                    lhsT=w_sb[:, co, t * NCI + ci],
                                    rhs=xp[ci][:, c * RG + kh:c * RG + kh + RG, kw:kw + W],
                                    start=(t == 0 and ci == 0),
                                    stop=(t == NTAP - 1 and ci == NCI - 1),
                                )
                        nc.scalar.activation(
                            out=osb[:, c * RG:(c + 1) * RG, :], in_=ps[:],
                            func=mybir.ActivationFunctionType.Identity,
                            bias=b_sb[:, co:co + 1], scale=s_sb[:, co:co + 1],
                        )
                    dst = o_d[n, co * 128:(co + 1) * 128]
                    nc.sync.dma_start(out=dst[:, 0:32], in_=osb[:, 0:32])
                    nc.sync.dma_start(out=dst[:, 32:H], in_=osb[:, 32:H])

    nc.compile()
    return nc


def _prep_weights(w_q, s, bias):
    # lhsT layout: [cin_k (128 partitions), co, (tap, ci), cout_j] so that
    # w_t[k, co, t*2+ci, j] = w_q[co*128 + j, ci*128 + k, kh, kw]
    w_t = (
        w_q.astype(np.float32)
        .transpose(2, 3, 1, 0)                 # [kh, kw, CIN, COUT]
        .reshape(NTAP, NCI, 128, NCO, 128)     # [tap, ci, k, co, j]
        .transpose(2, 3, 0, 1, 4)              # [k, co, tap, ci, j]
        .reshape(128, NCO, NTAP * NCI, 128)
        .astype(ml_dtypes.bfloat16)
    )
    s_t = np.ascontiguousarray(s.reshape(NCO, 128).T.astype(np.float32))
    b_t = np.ascontiguousarray(bias.reshape(NCO, 128).T.astype(np.float32))
    return w_t, s_t, b_t


def kernel(x, w_q, s, bias):
    if "nc" not in _compiled:
        _compiled["nc"] = _build()
    nc = _compiled["nc"]

    w_t, s_t, b_t = _prep_weights(w_q, s, bias)
    x = np.ascontiguousarray(x, dtype=np.float32)
    core_ids = list(range(NCORES))
    in_maps = [
        {"x": x[i * NB:(i + 1) * NB], "w": w_t, "s": s_t, "b": b_t}
        for i in core_ids
    ]
    res = run_bass_kernel_spmd(nc, in_maps, core_ids)
    return np.concatenate([res.results[i]["out"] for i in core_ids], axis=0)


# revision 3
# speedup vs baseline: 1.0421x; 1.0089x over previous
"""BitConv2dInfer on 8 Trainium2 NeuronCores.

Reference computation (per full input):
    x = clip(x, -1, 1)                       # x [32, 256, 56, 56] f32
    y = conv2d(x, w_q, pad=1)                # w_q [256, 256, 3, 3] ternary
    y = y * s + bias                         # per-out-channel affine
Sharding: data-parallel over batch — each of the 8 cores gets 4 images and
the full (tiny) weights; outputs concatenate over batch with no comms.

Device kernel (per core, per image):
  - DMA x[n] in as 2 CIN tiles of [128, 56, 56] f32 (row-chunked for the
    first image so the PE can start before the full image lands; DMA issue
    is spread across engines since each dma_start costs ~0.6us of issue time)
  - clamp to [-1,1] + cast to bf16 into a zero-bordered [128, 58, 58] pad tile
  - conv as 18 accumulated PE matmuls per (cout_tile, 8-row chunk):
      psum[co*128+m, oh, ow] += sum_k w[k, co*128+m] * xpad[k, oh+kh, ow+kw]
    lhsT = w slice [128 cin, 128 cout], rhs = shifted pad-tile window [128, 8, 56]
  - scalar-engine activation evacuates PSUM with per-partition scale+bias
  - DMA f32 result tiles back out (finely chunked for the last image so the
    tail drains early)

Weights are host-side transposed to lhsT layout [128 cin, co, (tap, ci), cout]
and cast to bf16 (exact for ternary values).
"""

import sys

sys.path.insert(0, "/opt/trn_rl_repo")

import ml_dtypes
import numpy as np

import concourse.bass as bass  # noqa: F401  (registers engines)
import concourse.mybir as mybir
import concourse.tile as tile
from concourse import bacc
from concourse.bass_utils import run_bass_kernel_spmd

N, CIN, COUT, H, W = 32, 256, 256, 56, 56
NCORES = 8
NB = N // NCORES          # images per core
HP, WP = H + 2, W + 2     # padded spatial
RG = 8                    # output rows per PSUM chunk (8*56=448 <= 512 f32/bank)
NCH = H // RG             # chunks per image
NCI = CIN // 128          # cin tiles
NCO = COUT // 128         # cout tiles
NTAP = 9
DRG = 16                  # input rows per first-image DMA chunk

_compiled = {}


def _build():
    nc = bacc.Bacc("TRN2", target_bir_lowering=False, debug=False)
    f32, bf16 = mybir.dt.float32, mybir.dt.bfloat16
    x_d = nc.dram_tensor("x", [NB, CIN, H, W], f32, kind="ExternalInput").ap()
    w_d = nc.dram_tensor(
        "w", [128, NCO, NTAP * NCI, 128], bf16, kind="ExternalInput"
    ).ap()
    s_d = nc.dram_tensor("s", [128, NCO], f32, kind="ExternalInput").ap()
    b_d = nc.dram_tensor("b", [128, NCO], f32, kind="ExternalInput").ap()
    o_d = nc.dram_tensor("out", [NB, COUT, H, W], f32, kind="ExternalOutput").ap()

    clamp = dict(op0=mybir.AluOpType.max, op1=mybir.AluOpType.min)

    with tile.TileContext(nc) as tc:
        with (
            tc.tile_pool(name="const", bufs=1) as cpool,
            tc.tile_pool(name="xs", bufs=3) as xspool,
            tc.tile_pool(name="xsc", bufs=3) as xscpool,
            tc.tile_pool(name="xpad", bufs=2) as xppool,
            tc.tile_pool(name="osb", bufs=3) as opool,
            tc.tile_pool(name="ps", bufs=6, space="PSUM") as pspool,
        ):
            w_sb = cpool.tile([128, NCO, NTAP * NCI, 128], bf16, tag="w")
            s_sb = cpool.tile([128, NCO], f32, tag="s")
            b_sb = cpool.tile([128, NCO], f32, tag="b")
            # co=0 weights first: the first matmul groups need only them.
            nc.sync.dma_start(out=w_sb[:, 0], in_=w_d[:, 0])

            # First image, row-chunked: DMA in DRG-row pieces (issued on two
            # engines in parallel), clamp in RG-row pieces so matmul group c
            # (reading padded rows [8c, 8c+10)) starts as soon as rows land.
            n0_xp = []
            for ci in range(NCI):
                xpad = xppool.tile([128, HP, WP], bf16, tag=f"xpad{ci}")
                nc.vector.memset(xpad[:, 0:1, :], 0.0)
                nc.vector.memset(xpad[:, HP - 1:HP, :], 0.0)
                nc.vector.memset(xpad[:, 1:HP - 1, 0:1], 0.0)
                nc.vector.memset(xpad[:, 1:HP - 1, WP - 1:WP], 0.0)
                n0_xp.append(xpad)
            n0_stage = []
            for r0 in range(0, H, DRG):
                nr = min(DRG, H - r0)
                for ci, eng in ((0, nc.gpsimd), (1, nc.scalar)):
                    xs = xscpool.tile([128, DRG, W], f32, tag=f"xsc{ci}")
                    eng.dma_start(
                        out=xs[:, 0:nr],
                        in_=x_d[0, ci * 128:(ci + 1) * 128, r0:r0 + nr],
                    )
                    n0_stage.append((r0, nr, ci, xs))
            # Remaining constants after the critical first chunks are queued.
            nc.sync.dma_start(out=w_sb[:, 1], in_=w_d[:, 1])
            nc.sync.dma_start(out=s_sb[:], in_=s_d)
            nc.sync.dma_start(out=b_sb[:], in_=b_d)
            for r0, nr, ci, xs in n0_stage:
                for q0 in range(0, nr, RG):
                    nc.vector.tensor_scalar(
                        n0_xp[ci][:, r0 + q0 + 1:r0 + q0 + RG + 1, 1:W + 1],
                        xs[:, q0:q0 + RG], -1.0, 1.0, **clamp,
                    )

            for n in range(NB):
                if n == 0:
                    xp = n0_xp
                else:
                    xp = []
                    for ci, eng in ((0, nc.gpsimd), (1, nc.sync)):
                        xpad = xppool.tile([128, HP, WP], bf16, tag=f"xpad{ci}")
                        nc.vector.memset(xpad[:, 0:1, :], 0.0)
                        nc.vector.memset(xpad[:, HP - 1:HP, :], 0.0)
                        nc.vector.memset(xpad[:, 1:HP - 1, 0:1], 0.0)
                        nc.vector.memset(xpad[:, 1:HP - 1, WP - 1:WP], 0.0)
                        xs = xspool.tile([128, H, W], f32, tag="xs")
                        eng.dma_start(out=xs[:], in_=x_d[n, ci * 128:(ci + 1) * 128])
                        nc.vector.tensor_scalar(
                            xpad[:, 1:H + 1, 1:W + 1], xs[:], -1.0, 1.0, **clamp
                        )
                        xp.append(xpad)
                for co in range(NCO):
                    osb = opool.tile([128, H, W], f32, tag="osb")
                    for c in range(NCH):
                        ps = pspool.tile([128, RG, W], f32, tag="ps")
                        for t in range(NTAP):
                            kh, kw = divmod(t, 3)
                            for ci in range(NCI):
                                nc.tensor.matmul(
                                    out=ps[:],
                                    lhsT=w_sb[:, co, t * NCI + ci],
                                    rhs=xp[ci][:, c * RG + kh:c * RG + kh + RG, kw:kw + W],
                                    start=(t == 0 and ci == 0),
                                    stop=(t == NTAP - 1 and ci == NCI - 1),
                                )
                        nc.scalar.activation(
                            out=osb[:, c * RG:(c + 1) * RG, :], in_=ps[:],
                            func=mybir.ActivationFunctionType.Identity,
                            bias=b_sb[:, co:co + 1], scale=s_sb[:, co:co + 1],
                        )
                    dst = o_d[n, co * 128:(co + 1) * 128]
                    if n == NB - 1:
                        # Tail: drain per-chunk so the final DMA is small.
                        for c in range(NCH):
                            nc.sync.dma_start(
                                out=dst[:, c * RG:(c + 1) * RG],
                                in_=osb[:, c * RG:(c + 1) * RG],
                            )
                    else:
                        nc.sync.dma_start(out=dst[:, 0:32], in_=osb[:, 0:32])
                        nc.sync.dma_start(out=dst[:, 32:H], in_=osb[:, 32:H])

    nc.compile()
    return nc


def _prep_weights(w_q, s, bias):
    # lhsT layout: [cin_k (128 partitions), co, (tap, ci), cout_j] so that
    # w_t[k, co, t*2+ci, j] = w_q[co*128 + j, ci*128 + k, kh, kw]
    w_t = (
        w_q.astype(np.float32)
        .transpose(2, 3, 1, 0)                 # [kh, kw, CIN, COUT]
        .reshape(NTAP, NCI, 128, NCO, 128)     # [tap, ci, k, co, j]
        .transpose(2, 3, 0, 1, 4)              # [k, co, tap, ci, j]
        .reshape(128, NCO, NTAP * NCI, 128)
        .astype(ml_dtypes.bfloat16)
    )
    s_t = np.ascontiguousarray(s.reshape(NCO, 128).T.astype(np.float32))
    b_t = np.ascontiguousarray(bias.reshape(NCO, 128).T.astype(np.float32))
    return w_t, s_t, b_t


def kernel(x, w_q, s, bias):
    if "nc" not in _compiled:
        _compiled["nc"] = _build()
    nc = _compiled["nc"]

    w_t, s_t, b_t = _prep_weights(w_q, s, bias)
    x = np.ascontiguousarray(x, dtype=np.float32)
    core_ids = list(range(NCORES))
    in_maps = [
        {"x": x[i * NB:(i + 1) * NB], "w": w_t, "s": s_t, "b": b_t}
        for i in core_ids
    ]
    res = run_bass_kernel_spmd(nc, in_maps, core_ids)
    return np.concatenate([res.results[i]["out"] for i in core_ids], axis=0)
